# revision 4
# baseline (speedup 1.0000x reference)
"""GAT (2-layer, 4-head then 1-head) on 8 Trainium2 NeuronCores.

Strategy
--------
- Nodes are permuted: globally degree-sorted, dealt round-robin to 8 cores
  (edge balance + nearly-identical degree profiles per core), then each
  core's nodes are laid out in 128-node dst tiles. Tiles are degree-uniform,
  so per-dst edge lists pad to the tile max with tiny waste.
- Edges land in a "slot grid" [128 dst x K slots] per tile: slot-chunk c is
  128 edges whose partition IS the dst row. The aggregation matmul then has
  an identity stationary operand (no per-chunk one-hot masks at all).
- Per-edge messages are fetched with dma_gather (int16 indices). The node
  table is split at the core-5/6 row boundary so both halves fit in int16
  (rows 0..30720 via table A view, rows 30721.. via offset view). Sentinel
  rows (0 and last) have attention logits of -1e30 so padded slots get
  weight exp(-inf) = 0 and contribute nothing, including to the softmax
  denominator.
- Softmax is computed without the max-subtraction (values are O(1); the
  normalization cancels exactly): w_e = exp(leakyrelu(x)) = max(exp(x),
  exp(0.2 x)), accumulated per dst in PSUM along with the denominator, and
  divided once per node.
- Layer outputs are transformed (W2 / W_out) per tile; the layer-2 node
  table is AllGather'd across cores between layers (as is the layer-1
  table after the sharded x @ W1 phase).
"""

import numpy as np

import concourse.bacc as bacc
import concourse.mybir as mybir
import concourse.tile as tile
from concourse.bass_utils import run_bass_kernel_spmd

F32 = mybir.dt.float32
I16 = mybir.dt.int16

IN_CH = 128
HID = 32
HEADS = 4
OUT_CH = 112
NEG_SLOPE = 0.2
NEG_BIG = -1e30

# Layer-1 table row: [h(128) | al_src(4) | al_dst(4) | one(1) | pad] = 192 f32 (768B)
T1_COLS = 192
T1_USED = 137
# Layer-2 table row: [z(32) | as2(1) | ad2(1) | one(1) | pad] = 64 f32 (256B)
T2_COLS = 64
T2_USED = 35

N_CORES = 8
KCAP = 20


def _prep(x, edge_index, W1, a_src1, a_dst1, b1, W2, a_src2, a_dst2, b2, W_out, b_out):
    """Host-side graph preprocessing. Returns (meta, per-core inputs)."""
    N = x.shape[0]
    E = edge_index.shape[1]
    per_core = -(-N // (N_CORES * 128)) * 128
    n_pad = per_core * N_CORES
    NT = per_core // 128
    nrows = n_pad + 2  # + 2 sentinel rows
    b_base = 1 + 6 * per_core  # first table row owned by core 6
    assert b_base - 1 <= 32767 and nrows - b_base <= 32767

    src = np.concatenate([edge_index[0], np.arange(N, dtype=np.int64)])
    dst = np.concatenate([edge_index[1], np.arange(N, dtype=np.int64)])
    E2 = src.shape[0]

    deg = np.bincount(dst, minlength=n_pad)
    order = np.argsort(deg, kind="stable")  # ascending degree, pads first
    # deal round-robin: global rank i -> core i%8, position i//8
    rank = np.empty(n_pad, np.int64)
    rank[order] = np.arange(n_pad)
    core_of = rank % N_CORES
    pos_in_core = rank // N_CORES
    grow = core_of * per_core + pos_in_core      # global row-1 (0-based over n_pad)
    trow = 1 + grow                               # table row of each node
    # perm_rows[g] = node sitting at global row g
    perm_rows = np.empty(n_pad, np.int64)
    perm_rows[grow] = np.arange(n_pad)

    sr = trow[src]
    dr = grow[dst]
    grp = (sr >= b_base).astype(np.int64)  # 0 = A half, 1 = B half

    # sort edges by (dst row, group); compute slot index within each run
    eorder = np.lexsort((grp, dr))
    dr_s = dr[eorder]
    sr_s = sr[eorder]
    g_s = grp[eorder]
    key = dr_s * 2 + g_s
    newrun = np.empty(E2, bool)
    newrun[0] = True
    newrun[1:] = key[1:] != key[:-1]
    run_id = np.cumsum(newrun) - 1
    run_start = np.flatnonzero(newrun)
    slot = np.arange(E2) - run_start[run_id]

    a_cnt = np.bincount(dr_s[g_s == 0], minlength=n_pad)
    b_cnt = np.bincount(dr_s[g_s == 1], minlength=n_pad)
    # per-(core,tile) maxima, unified across cores
    Ka = a_cnt.reshape(N_CORES, NT, 128).max(axis=(0, 2))
    Kb = b_cnt.reshape(N_CORES, NT, 128).max(axis=(0, 2))
    Kt = Ka + Kb

    # idx stream layout: per tile, A block then B block, slot-major
    base_a = np.zeros(NT, np.int64)
    base_b = np.zeros(NT, np.int64)
    off = 0
    for t in range(NT):
        base_a[t] = off
        off += 128 * Ka[t]
        base_b[t] = off
        off += 128 * Kb[t]
    totidx = off
    sent_b_local = nrows - 1 - b_base

    # default stream = sentinels
    default = np.zeros(totidx, np.int16)
    for t in range(NT):
        default[base_a[t]:base_a[t] + 128 * Ka[t]] = 0
        default[base_b[t]:base_b[t] + 128 * Kb[t]] = sent_b_local
    streams = np.tile(default, (N_CORES, 1))

    e_core = dr_s // per_core
    loc = dr_s % per_core
    tl = loc // 128
    p = loc % 128
    posA = base_a[tl] + slot * 128 + p
    posB = base_b[tl] + slot * 128 + p
    pos = np.where(g_s == 0, posA, posB)
    val = np.where(g_s == 0, sr_s, sr_s - b_base).astype(np.int16)
    streams[e_core, pos] = val

    # wrap for dma_gather: wrapped[p, j] = flat[j*16 + p%16]
    assert totidx % 16 == 0
    idx_wrapped = np.empty((N_CORES, 128, totidx // 16), np.int16)
    for c in range(N_CORES):
        w16 = streams[c].reshape(-1, 16).T  # [16, totidx/16]
        idx_wrapped[c] = np.tile(w16, (8, 1))

    # x slices (table-row order per core)
    xp = np.zeros((n_pad, IN_CH), np.float32)
    xp[:N] = np.asarray(x, np.float32)
    x_slices = np.empty((N_CORES, IN_CH, per_core), np.float32)
    for c in range(N_CORES):
        nodes = perm_rows[c * per_core:(c + 1) * per_core]
        x_slices[c] = xp[nodes].T

    # weight packs
    W1 = np.asarray(W1, np.float32)
    Bsrc = np.zeros((HEADS * HID, HEADS), np.float32)
    Bdst = np.zeros((HEADS * HID, HEADS), np.float32)
    for h in range(HEADS):
        Bsrc[h * HID:(h + 1) * HID, h] = np.asarray(a_src1[h], np.float32)
        Bdst[h * HID:(h + 1) * HID, h] = np.asarray(a_dst1[h], np.float32)
    W1big = np.concatenate([W1, W1 @ Bsrc, W1 @ Bdst], axis=1)  # [128, 136]
    W2 = np.asarray(W2, np.float32)
    W2big = np.concatenate(
        [W2, W2 @ np.asarray(a_src2, np.float32).T, W2 @ np.asarray(a_dst2, np.float32).T],
        axis=1,
    )  # [128, 34]
    b1_rep = np.tile(np.asarray(b1, np.float32)[None, :], (128, 1))          # [128,128]
    b2_rep = np.zeros((128, HID + 2), np.float32)
    b2_rep[:, :HID] = np.asarray(b2, np.float32)[None, :]
    bout_rep = np.tile(np.asarray(b_out, np.float32)[None, :], (128, 1))     # [128,112]
    ident = np.eye(128, dtype=np.float32)

    sent1 = np.zeros((1, 144), np.float32)
    sent1[0, 128:136] = NEG_BIG
    sent2 = np.zeros((1, 36), np.float32)
    sent2[0, 32:34] = NEG_BIG

    meta = dict(
        N=N, E2=E2, n_pad=n_pad, per_core=per_core, NT=NT, nrows=nrows,
        b_base=b_base, Ka=Ka.tolist(), Kb=Kb.tolist(),
        base_a=base_a.tolist(), base_b=base_b.tolist(), totidx=totidx,
        perm_rows=perm_rows,
    )
    shared = dict(
        W1big=W1big, W2big=W2big, Wout=np.asarray(W_out, np.float32),
        b1_rep=b1_rep, b2_rep=b2_rep, bout_rep=bout_rep, ident=ident,
        sent1=sent1, sent2=sent2,
    )
    in_maps = []
    for c in range(N_CORES):
        m = dict(shared)
        m["x_slice"] = np.ascontiguousarray(x_slices[c])
        m["idx_flat"] = np.ascontiguousarray(idx_wrapped[c])
        in_maps.append(m)
    return meta, in_maps


def _build(meta):
    per_core, NT, nrows, b_base = meta["per_core"], meta["NT"], meta["nrows"], meta["b_base"]
    Ka, Kb = meta["Ka"], meta["Kb"]
    base_a, base_b, totidx = meta["base_a"], meta["base_b"], meta["totidx"]

    nc = bacc.Bacc("TRN2", num_devices=N_CORES, num_swdge_queues=4,
                   dynamic_dma_scratch_size=65536)

    x_slice = nc.dram_tensor("x_slice", [IN_CH, per_core], F32, kind="ExternalInput")
    idx_flat = nc.dram_tensor("idx_flat", [128, totidx // 16], I16, kind="ExternalInput")
    W1big_d = nc.dram_tensor("W1big", [128, 136], F32, kind="ExternalInput")
    W2big_d = nc.dram_tensor("W2big", [128, HID + 2], F32, kind="ExternalInput")
    Wout_d = nc.dram_tensor("Wout", [HID, OUT_CH], F32, kind="ExternalInput")
    b1_d = nc.dram_tensor("b1_rep", [128, 128], F32, kind="ExternalInput")
    b2_d = nc.dram_tensor("b2_rep", [128, HID + 2], F32, kind="ExternalInput")
    bout_d = nc.dram_tensor("bout_rep", [128, OUT_CH], F32, kind="ExternalInput")
    ident_d = nc.dram_tensor("ident", [128, 128], F32, kind="ExternalInput")
    sent1_d = nc.dram_tensor("sent1", [1, 144], F32, kind="ExternalInput")
    sent2_d = nc.dram_tensor("sent2", [1, 36], F32, kind="ExternalInput")

    T1_own = nc.dram_tensor("T1_own", [per_core, T1_COLS], F32, kind="Internal")
    T1_sh = nc.dram_tensor("T1_sh", [nrows, T1_COLS], F32, kind="Internal", addr_space="Shared")
    T2_own = nc.dram_tensor("T2_own", [per_core, T2_COLS], F32, kind="Internal")
    T2_sh = nc.dram_tensor("T2_sh", [nrows, T2_COLS], F32, kind="Internal", addr_space="Shared")
    out_d = nc.dram_tensor("out", [per_core, OUT_CH], F32, kind="ExternalOutput")

    groups = [list(range(N_CORES))]
    qctr = [0]

    def qn():
        q = qctr[0] % 4
        qctr[0] += 1
        return q

    with tile.TileContext(nc) as tc:
        with (
            tc.tile_pool(name="const", bufs=1) as cp,
            tc.tile_pool(name="xa", bufs=2) as xap,
            tc.tile_pool(name="stage", bufs=2) as sp,
            tc.tile_pool(name="g1", bufs=2) as g1p,
            tc.tile_pool(name="g2", bufs=2) as g2p,
            tc.tile_pool(name="small", bufs=3) as smp,
            tc.tile_pool(name="rhs", bufs=2) as rp,
            tc.tile_pool(name="epi", bufs=2) as ep,
            tc.tile_pool(name="ps", bufs=2, space="PSUM") as pp,
        ):
            # ---- consts to SBUF
            W1big = cp.tile([128, 136], F32)
            nc.sync.dma_start(out=W1big[:], in_=W1big_d[:])
            W2big = cp.tile([128, HID + 2], F32)
            nc.sync.dma_start(out=W2big[:], in_=W2big_d[:])
            Wout = cp.tile([HID, OUT_CH], F32)
            nc.sync.dma_start(out=Wout[:], in_=Wout_d[:])
            b1r = cp.tile([128, 128], F32)
            nc.sync.dma_start(out=b1r[:], in_=b1_d[:])
            b2r = cp.tile([128, HID + 2], F32)
            nc.sync.dma_start(out=b2r[:], in_=b2_d[:])
            boutr = cp.tile([128, OUT_CH], F32)
            nc.sync.dma_start(out=boutr[:], in_=bout_d[:])
            ident = cp.tile([128, 128], F32)
            nc.sync.dma_start(out=ident[:], in_=ident_d[:])
            idxs = cp.tile([128, totidx // 16], I16)
            nc.sync.dma_start(out=idxs[:], in_=idx_flat[:])

            # ---- phase A: own node tiles -> T1_own
            for t in range(NT):
                xa = xap.tile([128, 128], F32)
                nc.sync.dma_start(out=xa[:], in_=x_slice[:, t * 128:(t + 1) * 128])
                ps = pp.tile([128, 136], F32, tag="agg")
                nc.tensor.matmul(out=ps[:], lhsT=xa[:], rhs=W1big[:], start=True, stop=True)
                hb = sp.tile([128, T1_USED], F32)
                nc.vector.tensor_copy(out=hb[:, 0:136], in_=ps[:])
                nc.vector.memset(hb[:, 136:137], 1.0)
                nc.sync.dma_start(out=T1_own[t * 128:(t + 1) * 128, 0:T1_USED], in_=hb[:])

            # ---- allgather T1 + sentinel pokes
            nc.gpsimd.collective_compute(
                "AllGather", mybir.AluOpType.bypass, replica_groups=groups,
                ins=[T1_own[:]], outs=[T1_sh[1:1 + N_CORES * per_core, :]],
            )
            s1 = cp.tile([1, 144], F32)
            nc.sync.dma_start(out=s1[:], in_=sent1_d[:])
            nc.sync.dma_start(out=T1_sh[0:1, 0:144], in_=s1[:])
            nc.sync.dma_start(out=T1_sh[nrows - 1:nrows, 0:144], in_=s1[:])

            # ---- layer 1 edge phase
            for t in range(NT):
                ka, kb = Ka[t], Kb[t]
                kt = ka + kb
                if kt == 0:
                    continue
                al8 = smp.tile([128, 8], F32, tag="al8")
                nc.sync.dma_start(out=al8[:], in_=T1_own[t * 128:(t + 1) * 128, 128:136])
                ps = pp.tile([128, HEADS * (HID + 1)], F32, tag="agg")
                for s0 in range(0, kt, KCAP):
                    s1e = min(s0 + KCAP, kt)
                    kp = s1e - s0
                    G = g1p.tile([128, kp, T1_COLS], F32, tag="G1")
                    na = min(s1e, ka) - min(s0, ka)   # A slots in this pass
                    nb = kp - na
                    if na:
                        o = base_a[t] + s0 * 128
                        nc.gpsimd.dma_gather(
                            G[:, 0:na, :], T1_sh[:], idxs[:, o // 16: o // 16 + na * 8],
                            128 * na, 128 * na, T1_COLS,
                            queue_num=qn(), single_packet=False)
                    if nb:
                        o = base_b[t] + max(s0 - ka, 0) * 128
                        nc.gpsimd.dma_gather(
                            G[:, na:kp, :], T1_sh[b_base:nrows, :], idxs[:, o // 16: o // 16 + nb * 8],
                            128 * nb, 128 * nb, T1_COLS,
                            queue_num=qn(), single_packet=False)
                    # logits x = as[src] + ad[dst]
                    xl = smp.tile([128, kp, HEADS], F32, tag="xl")
                    nc.vector.tensor_tensor(
                        out=xl[:], in0=G[:, :, 128:132],
                        in1=al8[:, None, 4:8].to_broadcast([128, kp, HEADS]),
                        op=mybir.AluOpType.add)
                    e1 = smp.tile([128, kp, HEADS], F32, tag="e1")
                    nc.scalar.activation(e1[:], xl[:], mybir.ActivationFunctionType.Exp)
                    e2 = smp.tile([128, kp, HEADS], F32, tag="e2")
                    nc.scalar.activation(e2[:], xl[:], mybir.ActivationFunctionType.Exp, scale=NEG_SLOPE)
                    w = smp.tile([128, kp, HEADS], F32, tag="w")
                    nc.vector.tensor_tensor(out=w[:], in0=e1[:], in1=e2[:], op=mybir.AluOpType.max)
                    # rhs = [w_h * h_h | w_h] per head
                    rhs = rp.tile([128, kp, HEADS * (HID + 1)], F32, tag="rhs1")
                    rhs_v = rhs[:].rearrange("p k (h j) -> p k h j", h=HEADS)
                    nc.vector.tensor_tensor(
                        out=rhs_v[:, :, :, 0:HID],
                        in0=G[:, :, 0:128].rearrange("p k (h j) -> p k h j", h=HEADS),
                        in1=w[:, :, :, None].to_broadcast([128, kp, HEADS, HID]),
                        op=mybir.AluOpType.mult)
                    nc.vector.tensor_copy(out=rhs_v[:, :, :, HID:HID + 1], in_=w[:, :, :, None])
                    for cch in range(kp):
                        nc.tensor.matmul(out=ps[:], lhsT=ident[:], rhs=rhs[:, cch, :],
                                         start=(s0 == 0 and cch == 0),
                                         stop=(s1e == kt and cch == kp - 1))
                # epilogue: divide, +b1, ELU
                ps_v = ps[:].rearrange("p (h j) -> p h j", h=HEADS)
                rec = smp.tile([128, HEADS], F32, tag="rec")
                nc.vector.reciprocal(out=rec[:], in_=ps_v[:, :, HID])
                y1 = ep.tile([128, 128], F32, tag="y1")
                nc.vector.tensor_tensor(
                    out=y1[:].rearrange("p (h j) -> p h j", h=HEADS),
                    in0=ps_v[:, :, 0:HID],
                    in1=rec[:, :, None].to_broadcast([128, HEADS, HID]),
                    op=mybir.AluOpType.mult)
                nc.vector.tensor_tensor(out=y1[:], in0=y1[:], in1=b1r[:], op=mybir.AluOpType.add)
                m1 = ep.tile([128, 128], F32, tag="m1")
                nc.vector.tensor_scalar(out=m1[:], in0=y1[:], scalar1=0.0, scalar2=None,
                                        op0=mybir.AluOpType.min)
                eE = ep.tile([128, 128], F32, tag="eE")
                nc.scalar.activation(eE[:], m1[:], mybir.ActivationFunctionType.Exp)
                r1 = ep.tile([128, 128], F32, tag="r1")
                nc.vector.tensor_scalar(out=r1[:], in0=y1[:], scalar1=0.0, scalar2=-1.0,
                                        op0=mybir.AluOpType.max, op1=mybir.AluOpType.add)
                h2 = ep.tile([128, 128], F32, tag="h2")
                nc.vector.tensor_tensor(out=h2[:], in0=eE[:], in1=r1[:], op=mybir.AluOpType.add)
                # transpose h2, z = h2 @ W2big
                pt = pp.tile([128, 128], F32, tag="pt")
                nc.tensor.transpose(out=pt[:], in_=h2[:], identity=ident[:])
                h2T = ep.tile([128, 128], F32, tag="h2T")
                nc.vector.tensor_copy(out=h2T[:], in_=pt[:])
                psz = pp.tile([128, HID + 2], F32, tag="z")
                nc.tensor.matmul(out=psz[:], lhsT=h2T[:], rhs=W2big[:], start=True, stop=True)
                t2b = sp.tile([128, T2_USED], F32, tag="t2b")
                nc.vector.tensor_tensor(out=t2b[:, 0:HID + 2], in0=psz[:], in1=b2r[:],
                                        op=mybir.AluOpType.add)
                nc.vector.memset(t2b[:, HID + 2:HID + 3], 1.0)
                nc.sync.dma_start(out=T2_own[t * 128:(t + 1) * 128, 0:T2_USED], in_=t2b[:])

            # ---- allgather T2 + sentinel pokes
            nc.gpsimd.collective_compute(
                "AllGather", mybir.AluOpType.bypass, replica_groups=groups,
                ins=[T2_own[:]], outs=[T2_sh[1:1 + N_CORES * per_core, :]],
            )
            s2 = cp.tile([1, 36], F32)
            nc.sync.dma_start(out=s2[:], in_=sent2_d[:])
            nc.sync.dma_start(out=T2_sh[0:1, 0:36], in_=s2[:])
            nc.sync.dma_start(out=T2_sh[nrows - 1:nrows, 0:36], in_=s2[:])

            # ---- layer 2 edge phase + output
            for t in range(NT):
                ka, kb = Ka[t], Kb[t]
                kt = ka + kb
                if kt == 0:
                    continue
                ad2 = smp.tile([128, 2], F32, tag="ad2")
                nc.sync.dma_start(out=ad2[:], in_=T2_own[t * 128:(t + 1) * 128, 32:34])
                ps2 = pp.tile([128, T2_USED], F32, tag="agg")
                for s0 in range(0, kt, KCAP):
                    s1e = min(s0 + KCAP, kt)
                    kp = s1e - s0
                    G2 = g2p.tile([128, kp, T2_COLS], F32, tag="G2")
                    na = min(s1e, ka) - min(s0, ka)
                    nb = kp - na
                    if na:
                        o = base_a[t] + s0 * 128
                        nc.gpsimd.dma_gather(
                            G2[:, 0:na, :], T2_sh[:], idxs[:, o // 16: o // 16 + na * 8],
                            128 * na, 128 * na, T2_COLS,
                            queue_num=qn(), single_packet=False)
                    if nb:
                        o = base_b[t] + max(s0 - ka, 0) * 128
                        nc.gpsimd.dma_gather(
                            G2[:, na:kp, :], T2_sh[b_base:nrows, :], idxs[:, o // 16: o // 16 + nb * 8],
                            128 * nb, 128 * nb, T2_COLS,
                            queue_num=qn(), single_packet=False)
                    xl2 = smp.tile([128, kp, 1], F32, tag="xl2")
                    nc.vector.tensor_tensor(
                        out=xl2[:], in0=G2[:, :, 32:33],
                        in1=ad2[:, None, 1:2].to_broadcast([128, kp, 1]),
                        op=mybir.AluOpType.add)
                    e1b = smp.tile([128, kp, 1], F32, tag="e1b")
                    nc.scalar.activation(e1b[:], xl2[:], mybir.ActivationFunctionType.Exp)
                    e2b = smp.tile([128, kp, 1], F32, tag="e2b")
                    nc.scalar.activation(e2b[:], xl2[:], mybir.ActivationFunctionType.Exp, scale=NEG_SLOPE)
                    w2 = smp.tile([128, kp, 1], F32, tag="w2")
                    nc.vector.tensor_tensor(out=w2[:], in0=e1b[:], in1=e2b[:], op=mybir.AluOpType.max)
                    rhs2 = rp.tile([128, kp, T2_USED], F32, tag="rhs2")
                    nc.vector.tensor_tensor(
                        out=rhs2[:], in0=G2[:, :, 0:T2_USED],
                        in1=w2[:, :, :].to_broadcast([128, kp, T2_USED]),
                        op=mybir.AluOpType.mult)
                    for cch in range(kp):
                        nc.tensor.matmul(out=ps2[:], lhsT=ident[:], rhs=rhs2[:, cch, :],
                                         start=(s0 == 0 and cch == 0),
                                         stop=(s1e == kt and cch == kp - 1))
                rec2 = smp.tile([128, 1], F32, tag="rec2")
                nc.vector.reciprocal(out=rec2[:], in_=ps2[:, HID + 2:HID + 3])
                y2 = ep.tile([128, HID], F32, tag="y2")
                nc.vector.tensor_tensor(
                    out=y2[:], in0=ps2[:, 0:HID],
                    in1=rec2[:].to_broadcast([128, HID]),
                    op=mybir.AluOpType.mult)
                m2 = ep.tile([128, HID], F32, tag="m2")
                nc.vector.tensor_scalar(out=m2[:], in0=y2[:], scalar1=0.0, scalar2=None,
                                        op0=mybir.AluOpType.min)
                eE2 = ep.tile([128, HID], F32, tag="eE2")
                nc.scalar.activation(eE2[:], m2[:], mybir.ActivationFunctionType.Exp)
                r2 = ep.tile([128, HID], F32, tag="r2")
                nc.vector.tensor_scalar(out=r2[:], in0=y2[:], scalar1=0.0, scalar2=-1.0,
                                        op0=mybir.AluOpType.max, op1=mybir.AluOpType.add)
                h3 = ep.tile([128, HID], F32, tag="h3")
                nc.vector.tensor_tensor(out=h3[:], in0=eE2[:], in1=r2[:], op=mybir.AluOpType.add)
                pt2 = pp.tile([128, 128], F32, tag="pt")
                nc.tensor.transpose(out=pt2[:HID, :], in_=h3[:], identity=ident[:])
                h3T = ep.tile([HID, 128], F32, tag="h3T")
                nc.vector.tensor_copy(out=h3T[:], in_=pt2[:HID, :])
                psf = pp.tile([128, OUT_CH], F32, tag="z")
                nc.tensor.matmul(out=psf[:], lhsT=h3T[:], rhs=Wout[:], start=True, stop=True)
                outf = ep.tile([128, OUT_CH], F32, tag="outf")
                nc.vector.tensor_tensor(out=outf[:], in0=psf[:], in1=boutr[:],
                                        op=mybir.AluOpType.add)
                nc.sync.dma_start(out=out_d[t * 128:(t + 1) * 128, :], in_=outf[:])

    nc.compile()
    return nc


def _run(inputs, trace=False):
    meta, in_maps = _prep(**inputs)
    nc = _build(meta)
    res = run_bass_kernel_spmd(nc, in_maps, core_ids=list(range(N_CORES)), trace=trace)
    per_core = meta["per_core"]
    outg = np.concatenate([res.results[c]["out"] for c in range(N_CORES)], axis=0)
    # global row g holds node perm_rows[g]
    out_nodes = np.empty((meta["n_pad"], OUT_CH), np.float32)
    out_nodes[meta["perm_rows"]] = outg
    return out_nodes[:meta["N"]], res


def kernel(**inputs):
    out, _ = _run(inputs, trace=False)
    return out
